# revision 2
# baseline (speedup 1.0000x reference)
"""BertLayer on 8 trn2 NeuronCores — data-parallel over batch (2 per core).

Compute-bound redesign:
  - fp8e4 DoubleRow matmuls (2x PE rate) for QKV/Wo/W1/W2 and the attention
    context; weights pre-scaled x64 on the host, descales folded into
    eviction ops. Residual path stays f32, decoupled from quantization.
  - Attention context in swapped orientation ctx_nat[q, d+1] so the softmax
    denominator is a PSUM column: reciprocal_approx_fast on [128,1] + a
    per-partition tensor_scalar, instead of serial [128,512] reciprocals and
    K=1 broadcast matmuls.
  - b-outer attention: batch-0 attention interleaves qT/kT production,
    batch-1 attention interleaves the Wo+LN1 work of batch-0's tokens.
  - PE transposes run one stage late (never waiting on DVE evictions) and
    pack two 128x128 tiles per PSUM bank (single accumulation group) so
    evictions are [128,256].
  - Engine budget: exp/gelu must live on scalar (the floor); evictions are
    spread scalar/vector; gpsimd does the SBUF-only residual ops.
"""

import sys

if "/opt/trn_rl_repo" not in sys.path:
    sys.path.insert(0, "/opt/trn_rl_repo")

from contextlib import ExitStack

import ml_dtypes
import numpy as np

import concourse.bass as bass
import concourse.tile as tile
from concourse import bacc, mybir
from concourse.masks import make_identity
from concourse.bass_utils import run_bass_kernel_spmd

F32 = mybir.dt.float32
BF16 = mybir.dt.bfloat16
F8 = mybir.dt.float8e4
AF = mybir.ActivationFunctionType
ALU = mybir.AluOpType
DR = mybir.MatmulPerfMode.DoubleRow

# ---- precision config ----
FP8_QKV = True
FP8_WO = True
FP8_W1 = True
FP8_W2 = True
FP8_CTX = True

# Problem dims (hardcoded: nn_BertLayer, hidden 768, 12 heads, ff 3072)
NB = 16
NCORES = 8
BPC = NB // NCORES
S = 512
T = BPC * S
H = 768
HK = H // 128
NH = 12
HD = 64
FF = 3072
EPS = 1e-12
MT = T // 128
NQ = 2           # ffn chunks
FQ = FF // NQ    # ff features per chunk
QK = FQ // 128   # k-tiles per chunk
SCALE = 1.0 / float(np.sqrt(HD))

WS = 64.0                      # host-side weight scale for fp8 weights
VS = WS if FP8_QKV else 1.0    # scale carried by q/k/v psums
CS = 32.0 if FP8_WO else 1.0   # scale of ctxT values
C1 = VS / CS                   # vA ones-column value
ESC = SCALE / (VS * VS)        # exp input scale
WO_DESC = 1.0 / (CS * (WS if FP8_WO else 1.0))
W1_DESC = 1.0 / WS if FP8_W1 else 1.0
W2_DESC = 1.0 / WS if FP8_W2 else 1.0

DT_X = F8 if FP8_QKV else BF16
DT_WQKV = F8 if FP8_QKV else BF16
DT_CTX = F8 if FP8_WO else BF16
DT_WO = F8 if FP8_WO else BF16
DT_H = F8 if FP8_W1 else BF16
DT_W1 = F8 if FP8_W1 else BF16
DT_G = F8 if FP8_W2 else BF16
DT_W2 = F8 if FP8_W2 else BF16
DT_E = F8 if FP8_CTX else BF16


def _bcast_row_ap(vec_ext, n):
    a = vec_ext[:]
    return bass.AP(tensor=a.tensor, offset=a.offset, ap=[[0, 128], [1, n]])


def _col_ap(vec_ext, ntiles):
    a = vec_ext[:]
    return bass.AP(tensor=a.tensor, offset=a.offset, ap=[[1, 128], [128, ntiles]])


def build_nc(ln1_trivial, ln2_trivial, bo_zero, bv_zero, bqk_zero):
    nc = bacc.Bacc(num_swdge_queues=4)

    x_ext = nc.declare_dram_parameter("hidden_state", [T, H], F32, isOutput=False)
    wq_e = nc.declare_dram_parameter("Wq", [H, H], DT_WQKV, isOutput=False)
    bq_e = nc.declare_dram_parameter("bq", [H], F32, isOutput=False)
    wk_e = nc.declare_dram_parameter("Wk", [H, H], DT_WQKV, isOutput=False)
    bk_e = nc.declare_dram_parameter("bk", [H], F32, isOutput=False)
    wv_e = nc.declare_dram_parameter("Wv", [H, H], DT_WQKV, isOutput=False)
    bv_e = nc.declare_dram_parameter("bv", [H], F32, isOutput=False)
    wo_e = nc.declare_dram_parameter("Wo", [H, H], DT_WO, isOutput=False)
    bo_e = nc.declare_dram_parameter("bo", [H], F32, isOutput=False)
    l1g_e = nc.declare_dram_parameter("ln1_g", [H], F32, isOutput=False)
    l1b2_e = nc.declare_dram_parameter("ln1b_plus_b2", [H], F32, isOutput=False)
    w1_e = nc.declare_dram_parameter("W1g", [H, FF], DT_W1, isOutput=False)
    b1_e = nc.declare_dram_parameter("b1f", [FF], F32, isOutput=False)
    w2_e = nc.declare_dram_parameter("W2", [FF, H], DT_W2, isOutput=False)
    l2g_e = nc.declare_dram_parameter("ln2_g", [H], F32, isOutput=False)
    l2b_e = nc.declare_dram_parameter("ln2_b", [H], F32, isOutput=False)
    out_ext = nc.declare_dram_parameter("out", [T, H], F32, isOutput=True)

    with ExitStack() as top:
        tc = top.enter_context(tile.TileContext(nc))

        const = top.enter_context(tc.tile_pool(name="const", bufs=1))
        small = top.enter_context(tc.tile_pool(name="small", bufs=4))
        psS = top.enter_context(tc.tile_pool(name="psS", bufs=4, space="PSUM"))
        psA = top.enter_context(tc.tile_pool(name="psA", bufs=2, space="PSUM"))
        ps_ctx = top.enter_context(tc.tile_pool(name="ps_ctx", bufs=2, space="PSUM"))
        main = top.enter_context(tc.tile_pool(name="main", bufs=1))
        wpool = top.enter_context(tc.tile_pool(name="wpool", bufs=3))
        outp = top.enter_context(tc.tile_pool(name="outp", bufs=3))

        ident = const.tile([128, 128], BF16, name="ident")
        make_identity(nc, ident)
        eps_col = const.tile([128, 1], F32, name="eps_col")
        nc.vector.memset(eps_col, EPS)

        bv_bc = const.tile([128, H], F32, name="bv_bc")
        nc.gpsimd.dma_start(out=bv_bc, in_=_bcast_row_ap(bv_e, H))
        bo_bc = const.tile([128, H], F32, name="bo_bc")
        nc.gpsimd.dma_start(out=bo_bc, in_=_bcast_row_ap(bo_e, H))
        l1g_bc = const.tile([128, H], F32, name="l1g_bc")
        nc.gpsimd.dma_start(out=l1g_bc, in_=_bcast_row_ap(l1g_e, H))
        lb2_bc = const.tile([128, H], F32, name="lb2_bc")
        nc.gpsimd.dma_start(out=lb2_bc, in_=_bcast_row_ap(l1b2_e, H))
        l2g_bc = const.tile([128, H], F32, name="l2g_bc")
        nc.gpsimd.dma_start(out=l2g_bc, in_=_bcast_row_ap(l2g_e, H))
        l2b_bc = const.tile([128, H], F32, name="l2b_bc")
        nc.gpsimd.dma_start(out=l2b_bc, in_=_bcast_row_ap(l2b_e, H))

        bq_cols = const.tile([128, HK], F32, name="bq_cols")
        nc.gpsimd.dma_start(out=bq_cols, in_=_col_ap(bq_e, HK))
        bk_cols = const.tile([128, HK], F32, name="bk_cols")
        nc.gpsimd.dma_start(out=bk_cols, in_=_col_ap(bk_e, HK))
        b1_cols = const.tile([128, FF // 128], F32, name="b1_cols")
        nc.gpsimd.dma_start(out=b1_cols, in_=_col_ap(b1_e, FF // 128))

        # -------- persistent tensors (slots recycled via tags) --------
        xT = main.tile([128, HK, T], DT_X, tag="s1", name="xT")
        ctxT = main.tile([128, HK, T], DT_CTX, tag="s2", name="ctxT")
        qT = main.tile([128, HK, T], BF16, tag="s3", bufs=2, name="qT")
        kT = main.tile([128, HK, T], BF16, tag="s4", name="kT")
        vA = main.tile([128, MT, NH, HD + 2], DT_E, tag="va", name="vA")
        nc.vector.memset(vA[:, :, :, HD:HD + 1], C1)
        x_nat = main.tile([128, MT, H], F32, tag="s5", name="x_nat")

        def acc_mm2(outs, lhs_fn, rhs_fns, nk, fp8):
            """K-accumulate into several psum groups; each stationary (lhsT)
            tile is used for all outs consecutively (weight reuse)."""
            if fp8:
                npair = nk // 2
                for i in range(npair):
                    for o, rf in zip(outs, rhs_fns):
                        nc.tensor.matmul(
                            o, lhs_fn(2 * i, 2), rf(2 * i, 2),
                            start=(i == 0), stop=(i == npair - 1),
                            perf_mode=DR)
            else:
                for kk in range(nk):
                    for o, rf in zip(outs, rhs_fns):
                        nc.tensor.matmul(
                            o, lhs_fn(kk, 1), rf(kk, 1),
                            start=(kk == 0), stop=(kk == nk - 1))

        def tr_pair(pool, tag, src_fn, dst_fn, npairs, evict_engines):
            """2 PE transposes share one PSUM bank (one accumulation group),
            evicted as a single [128,2,128] copy. dst_fn(p) must return a
            [128,2,128]-shaped AP."""
            for p in range(npairs):
                pt = pool.tile([128, 512], F32, tag=tag, name=f"tp{tag}")
                ptv = pt[:].bitcast(BF16)[:, 0:256]
                for j in range(2):
                    nc.tensor.matmul(
                        ptv[:, j * 128:(j + 1) * 128], src_fn(2 * p + j),
                        ident, is_transpose=True,
                        start=(j == 0), stop=(j == 1))
                src3 = ptv.rearrange("p (a b) -> p a b", a=2)
                eng = evict_engines[p % len(evict_engines)]
                if eng is nc.scalar:
                    nc.scalar.copy(out=dst_fn(p), in_=src3)
                else:
                    eng.tensor_copy(out=dst_fn(p), in_=src3)

        # ---------------- x load + transpose + V ----------------
        with ExitStack() as ph_ab:
            xload = ph_ab.enter_context(tc.tile_pool(name="xload", bufs=3))
            expp = ph_ab.enter_context(tc.tile_pool(name="expp", bufs=6))
            bcp = ph_ab.enter_context(tc.tile_pool(name="bcp", bufs=3))

            # x tiles first on the sync queue so transposes start ASAP
            xws = []
            for mt in range(MT):
                xw = xload.tile([128, H], F32, tag="xw", bufs=3, name="xw")
                nc.sync.dma_start(out=xw, in_=x_ext[mt * 128:(mt + 1) * 128, :])
                xws.append(xw)
            wvsb = wpool.tile([128, HK, H], DT_WQKV, tag="wsb", name="wvsb")
            for kk in range(HK):
                nc.sync.dma_start(
                    out=wvsb[:, kk, :], in_=wv_e[kk * 128:(kk + 1) * 128, :])

            for mt in range(MT):
                xwb = xload.tile([128, H], BF16, tag="xwb", name="xwb")
                nc.vector.tensor_copy(out=xwb, in_=xws[mt])
                tr_pair(
                    psS, "pss",
                    lambda c: xwb[:, c * 128:(c + 1) * 128],
                    lambda p: xT[:, 2 * p:2 * p + 2, mt * 128:(mt + 1) * 128],
                    HK // 2, [nc.scalar, nc.vector])

            def v_tile(mt):
                psv = [psA.tile([128, 512], F32, tag="ps", name="psv")
                       for _ in range(2)]
                acc_mm2(
                    [p[:, 0:384] for p in psv],
                    lambda i, w: xT[:, i:i + w, mt * 128:(mt + 1) * 128],
                    [(lambda i, w, n=n: wvsb[:, i:i + w, n * 384:(n + 1) * 384])
                     for n in range(2)],
                    HK, FP8_QKV)
                for nt2 in range(2):
                    dst = vA[:, mt, nt2 * 6:(nt2 + 1) * 6, 0:HD]
                    src = psv[nt2][:, 0:384].rearrange("p (h d) -> p h d", d=HD)
                    if bv_zero and nt2 == 0:
                        nc.scalar.copy(out=dst, in_=src)
                    elif bv_zero:
                        nc.vector.tensor_copy(out=dst, in_=src)
                    else:
                        nc.vector.tensor_add(
                            out=dst, in0=src,
                            in1=bv_bc[:, nt2 * 384:(nt2 + 1) * 384].rearrange(
                                "p (h d) -> p h d", d=HD))

            wqsb = wpool.tile([128, HK, H], DT_WQKV, tag="wsb", name="wqsb")
            for kk in range(HK):
                nc.sync.dma_start(
                    out=wqsb[:, kk, :], in_=wq_e[kk * 128:(kk + 1) * 128, :])
            wksb = wpool.tile([128, HK, H], DT_WQKV, tag="wsb", name="wksb")
            for kk in range(HK):
                nc.sync.dma_start(
                    out=wksb[:, kk, :], in_=wk_e[kk * 128:(kk + 1) * 128, :])

            def qk_pair(t):
                """Produce qT/kT for head-pair t (hidden cols t*128..)."""
                for (wsb, b_cols, dstT) in ((wqsb, bq_cols, qT),
                                            (wksb, bk_cols, kT)):
                    pss = [psA.tile([128, 512], F32, tag="ps", name="psqk")
                           for _ in range(2)]
                    acc_mm2(
                        [p[:] for p in pss],
                        lambda i, w: wsb[:, i:i + w, t * 128:(t + 1) * 128],
                        [(lambda i, w, n=n: xT[:, i:i + w,
                                               n * 512:(n + 1) * 512])
                         for n in range(2)],
                        HK, FP8_QKV)
                    for nt in range(2):
                        dst = dstT[:, t, nt * 512:(nt + 1) * 512]
                        if bqk_zero:
                            nc.vector.tensor_copy(out=dst, in_=pss[nt][:])
                        else:
                            nc.vector.tensor_scalar(
                                out=dst, in0=pss[nt][:],
                                scalar1=b_cols[:, t:t + 1], scalar2=None,
                                op0=ALU.add)

            # transposes of the previous sub-unit run one stage late so the
            # PE never waits on the vector evictions that produce cpk
            pend = []

            def flush_pend():
                while pend:
                    t0, b0, cpk0 = pend.pop(0)
                    tr_pair(
                        psA, "ps",
                        lambda qt: cpk0[:, qt, :],
                        lambda p: ctxT[:, t0,
                                       b0 * 512 + p * 256:
                                       b0 * 512 + (p + 1) * 256].rearrange(
                                           "p (a b) -> p a b", a=2),
                        2, [nc.vector])

            def attend_one(t, b):
                """Attention for both heads of pair t, one batch."""
                expTs = [expp.tile([128, 4, 512], DT_E, tag="expT",
                                   name="expT") for _ in range(2)]
                for kt in range(4):
                    for hh in range(2):
                        poff = hh * 64
                        ps_s = psS.tile([128, 512], F32, tag="pss",
                                        name="ps_s")
                        nc.tensor.matmul(
                            ps_s,
                            kT[poff:poff + 64, t,
                               b * 512 + kt * 128: b * 512 + (kt + 1) * 128],
                            qT[poff:poff + 64, t, b * 512:(b + 1) * 512],
                            start=True, stop=True,
                        )
                        nc.scalar.activation(
                            expTs[hh][:, kt, :], ps_s[:],
                            AF.Exp, scale=float(ESC))
                cpk = bcp.tile([128, 4, 128], BF16, tag="cpk", bufs=2,
                               name="cpk")
                for qp in range(2):
                    # 4 ctx accumulation groups share one PSUM bank: start
                    # only on the very first matmul (it zeroes the whole
                    # bank), stop only on the very last.
                    ps_c = ps_ctx.tile([128, 512], F32, tag="ctx",
                                       name="ps_c")
                    view = ps_c[:, 0:4 * (HD + 1)].rearrange(
                        "p (j c) -> p j c", c=HD + 1)
                    idx = 0
                    for qt2 in range(2):
                        qt = qp * 2 + qt2
                        for hh in range(2):
                            h = 2 * t + hh
                            if FP8_CTX:
                                for ktp in range(2):
                                    nc.tensor.matmul(
                                        view[:, idx, :],
                                        expTs[hh][:, 2 * ktp:2 * ktp + 2,
                                                  qt * 128:(qt + 1) * 128],
                                        vA[:, b * 4 + 2 * ktp:
                                           b * 4 + 2 * ktp + 2, h, 0:HD + 1],
                                        start=(idx == 0 and ktp == 0),
                                        stop=(idx == 3 and ktp == 1),
                                        perf_mode=DR,
                                    )
                            else:
                                for kt in range(4):
                                    nc.tensor.matmul(
                                        view[:, idx, :],
                                        expTs[hh][:, kt,
                                                  qt * 128:(qt + 1) * 128],
                                        vA[:, b * 4 + kt, h, 0:HD + 1],
                                        start=(idx == 0 and kt == 0),
                                        stop=(idx == 3 and kt == 3),
                                    )
                            idx += 1
                    rec4 = bcp.tile([128, 4], F32, tag="rec", bufs=4,
                                    name="rec4")
                    nc.vector.reciprocal_approx_fast(
                        out=rec4, in_=view[:, :, HD])
                    idx = 0
                    for qt2 in range(2):
                        qt = qp * 2 + qt2
                        for hh in range(2):
                            nc.vector.tensor_scalar(
                                out=cpk[:, qt, hh * 64:(hh + 1) * 64],
                                in0=view[:, idx, 0:HD],
                                scalar1=rec4[:, idx:idx + 1], scalar2=None,
                                op0=ALU.mult)
                            idx += 1
                flush_pend()
                pend.append((t, b, cpk))

            # ---- Wo + residual + LN1 + h transpose (per token tile) ----
            hT = main.tile([128, HK, T], DT_H, tag="s1x", name="hT")
            acc = main.tile([128, MT, H], F32, tag="s4x", name="acc")
            attp = ph_ab.enter_context(tc.tile_pool(name="attp", bufs=3))
            wosb = wpool.tile([128, HK, H], DT_WO, tag="wsb", name="wosb")

            def wo_ln1(mt):
                attn = attp.tile([128, H], F32, tag="attn", name="attn")
                psw = [psA.tile([128, 512], F32, tag="ps", name="psw")
                       for _ in range(2)]
                acc_mm2(
                    [p[:, 0:384] for p in psw],
                    lambda i, w: ctxT[:, i:i + w, mt * 128:(mt + 1) * 128],
                    [(lambda i, w, n=n: wosb[:, i:i + w, n * 384:(n + 1) * 384])
                     for n in range(2)],
                    HK, FP8_WO)
                for nt2 in range(2):
                    nc.vector.scalar_tensor_tensor(
                        out=attn[:, nt2 * 384:(nt2 + 1) * 384],
                        in0=psw[nt2][:, 0:384], scalar=float(WO_DESC),
                        in1=x_nat[:, mt, nt2 * 384:(nt2 + 1) * 384],
                        op0=ALU.mult, op1=ALU.add)
                st = small.tile([128, 3, 6], F32, tag="lnst", bufs=8, name="st")
                for i in range(3):
                    nc.vector.bn_stats(out=st[:, i, :],
                                       in_=attn[:, i * 256:(i + 1) * 256])
                mv = small.tile([128, 2], F32, tag="lnmv", bufs=8, name="mv")
                nc.vector.bn_aggr(out=mv[:], in_=st[:])
                sd = small.tile([128, 1], F32, tag="lnsd", bufs=8, name="sd")
                nc.scalar.activation(sd[:], mv[:, 1:2], AF.Abs_reciprocal_sqrt,
                                     bias=eps_col[:])
                hb = attp.tile([128, H], BF16, tag="hb", name="hb")
                nc.vector.tensor_scalar(
                    out=hb[:], in0=attn[:], scalar1=mv[:, 0:1], scalar2=sd[:],
                    op0=ALU.subtract, op1=ALU.mult)
                tr_pair(
                    ps_ctx, "ctx",
                    lambda c: hb[:, c * 128:(c + 1) * 128],
                    lambda p: hT[:, 2 * p:2 * p + 2, mt * 128:(mt + 1) * 128],
                    HK // 2, [nc.scalar, nc.vector])
                # residual path: acc = z*g1 + (ln1_b + b2)
                if ln1_trivial:
                    nc.gpsimd.tensor_add(acc[:, mt, :], hb[:], lb2_bc[:])
                else:
                    nc.gpsimd.tensor_mul(acc[:, mt, :], hb[:], l1g_bc[:])
                    nc.gpsimd.tensor_add(acc[:, mt, :], acc[:, mt, :],
                                         lb2_bc[:])

            # batch-0 attention sweep: interleaves qk production and the
            # second-half V tiles (only batch-0's V tiles are needed first)
            qk_pair(0)
            for mt in range(4):
                v_tile(mt)
            for t in range(HK):
                attend_one(t, 0)
                if t + 1 < HK:
                    qk_pair(t + 1)
                if t < 4:
                    v_tile(4 + t)

            # Wo weights + x_nat residual loads
            for kk in range(HK):
                nc.sync.dma_start(
                    out=wosb[:, kk, :], in_=wo_e[kk * 128:(kk + 1) * 128, :])
            for mt in range(MT):
                nc.gpsimd.dma_start(
                    out=x_nat[:, mt, :], in_=x_ext[mt * 128:(mt + 1) * 128, :])
                if not bo_zero:
                    nc.gpsimd.tensor_add(
                        out=x_nat[:, mt, :], in0=x_nat[:, mt, :], in1=bo_bc[:])

            # batch-1 attention sweep, interleaved with batch-0's Wo/LN1
            for t in range(HK):
                attend_one(t, 1)
                if t >= 2:
                    wo_ln1(t - 2)
            flush_pend()
            for mt in range(4, MT):
                wo_ln1(mt)

            # ---------------- FFN ----------------
            for q in range(NQ):
                w1c = wpool.tile([128, HK, FQ], DT_W1, tag="wsb", name="w1c")
                for kk in range(HK):
                    nc.sync.dma_start(
                        out=w1c[:, kk, :],
                        in_=w1_e[kk * 128:(kk + 1) * 128, q * FQ:(q + 1) * FQ])
                w2c = wpool.tile([128, QK, H], DT_W2, tag="wsb", name="w2c")
                for kk in range(QK):
                    row = (q * QK + kk) * 128
                    nc.sync.dma_start(out=w2c[:, kk, :],
                                      in_=w2_e[row:row + 128, :])
                gT = main.tile([128, QK, T], DT_G, tag="s3", bufs=2, name="gT")
                for mo in range(QK):
                    psf = [psS.tile([128, 512], F32, tag="pss", name="psf1")
                           for _ in range(2)]
                    acc_mm2(
                        [p[:] for p in psf],
                        lambda i, w: w1c[:, i:i + w, mo * 128:(mo + 1) * 128],
                        [(lambda i, w, n=n: hT[:, i:i + w,
                                               n * 512:(n + 1) * 512])
                         for n in range(2)],
                        HK, FP8_W1)
                    for nt in range(2):
                        nc.scalar.activation(
                            gT[:, mo, nt * 512:(nt + 1) * 512], psf[nt][:],
                            AF.Gelu, scale=float(W1_DESC),
                            bias=b1_cols[:, q * QK + mo:q * QK + mo + 1])
                for mt in range(MT):
                    psf2 = [psA.tile([128, 512], F32, tag="ps", name="psf2")
                            for _ in range(2)]
                    acc_mm2(
                        [p[:, 0:384] for p in psf2],
                        lambda i, w: gT[:, i:i + w, mt * 128:(mt + 1) * 128],
                        [(lambda i, w, n=n: w2c[:, i:i + w,
                                                n * 384:(n + 1) * 384])
                         for n in range(2)],
                        QK, FP8_W2)
                    for nt2 in range(2):
                        if FP8_W2:
                            nc.vector.scalar_tensor_tensor(
                                out=acc[:, mt, nt2 * 384:(nt2 + 1) * 384],
                                in0=psf2[nt2][:, 0:384], scalar=float(W2_DESC),
                                in1=acc[:, mt, nt2 * 384:(nt2 + 1) * 384],
                                op0=ALU.mult, op1=ALU.add)
                        else:
                            nc.vector.tensor_add(
                                out=acc[:, mt, nt2 * 384:(nt2 + 1) * 384],
                                in0=acc[:, mt, nt2 * 384:(nt2 + 1) * 384],
                                in1=psf2[nt2][:, 0:384])
                    if q == NQ - 1:
                        # LN2 + store as soon as this token tile is final
                        src = acc[:, mt, :]
                        st = small.tile([128, 3, 6], F32, tag="lnst", bufs=8,
                                        name="st2")
                        for i in range(3):
                            nc.vector.bn_stats(
                                out=st[:, i, :],
                                in_=src[:, i * 256:(i + 1) * 256])
                        mv = small.tile([128, 2], F32, tag="lnmv", bufs=8,
                                        name="mv2")
                        nc.vector.bn_aggr(out=mv[:], in_=st[:])
                        sd = small.tile([128, 1], F32, tag="lnsd", bufs=8,
                                        name="sd2")
                        nc.scalar.activation(sd[:], mv[:, 1:2],
                                             AF.Abs_reciprocal_sqrt,
                                             bias=eps_col[:])
                        ot = outp.tile([128, H], F32, tag="ot", name="ot")
                        nc.vector.tensor_scalar(
                            out=ot[:], in0=src, scalar1=mv[:, 0:1],
                            scalar2=sd[:], op0=ALU.subtract, op1=ALU.mult)
                        if not ln2_trivial:
                            nc.gpsimd.tensor_mul(ot[:], ot[:], l2g_bc[:])
                            nc.gpsimd.tensor_add(ot[:], ot[:], l2b_bc[:])
                        nc.sync.dma_start(
                            out=out_ext[mt * 128:(mt + 1) * 128, :], in_=ot)

    nc.finalize()
    return nc


_NC = None
_NC_KEY = None


def _get_nc(key):
    global _NC, _NC_KEY
    if _NC is None or _NC_KEY != key:
        _NC = build_nc(*key)
        _NC_KEY = key
    return _NC


def run(inputs, trace=False):
    f32 = lambda n: np.ascontiguousarray(np.asarray(inputs[n], dtype=np.float32))

    def conv(a, fp8):
        dt = ml_dtypes.float8_e4m3 if fp8 else ml_dtypes.bfloat16
        s = WS if fp8 else 1.0
        return np.ascontiguousarray((s * a).astype(dt))

    hs = f32("hidden_state").reshape(NB, S, H)
    w1 = f32("W1")
    l1g = f32("ln1_g")
    l1b = f32("ln1_b")
    l1b2 = l1b + f32("b2")
    l2g = f32("ln2_g")
    l2b = f32("ln2_b")
    bo = f32("bo")
    ln1_trivial = bool(np.all(l1g == 1.0))
    ln2_trivial = bool(np.all(l2g == 1.0) and np.all(l2b == 0.0))
    bo_zero = bool(np.all(bo == 0.0))
    bv_zero = bool(np.all(f32("bv") == 0.0))
    bqk_zero = bool(np.all(f32("bq") == 0.0) and np.all(f32("bk") == 0.0))
    common = {
        "Wq": conv(f32("Wq"), FP8_QKV), "bq": VS * f32("bq"),
        "Wk": conv(f32("Wk"), FP8_QKV), "bk": VS * f32("bk"),
        "Wv": conv(f32("Wv"), FP8_QKV), "bv": VS * f32("bv"),
        "Wo": conv(f32("Wo"), FP8_WO), "bo": bo,
        "ln1_g": l1g,
        "ln1b_plus_b2": np.ascontiguousarray(l1b2),
        # fold LN1 gamma/beta into the FFN input projection
        "W1g": conv(l1g[:, None] * w1, FP8_W1),
        "b1f": np.ascontiguousarray(f32("b1") + l1b @ w1),
        "W2": conv(f32("W2"), FP8_W2),
        "ln2_g": l2g, "ln2_b": l2b,
    }
    in_maps = []
    for i in range(NCORES):
        m = dict(common)
        m["hidden_state"] = np.ascontiguousarray(
            hs[i * BPC:(i + 1) * BPC].reshape(T, H))
        in_maps.append(m)
    key = (ln1_trivial, ln2_trivial, bo_zero, bv_zero, bqk_zero)
    res = run_bass_kernel_spmd(_get_nc(key), in_maps,
                               core_ids=list(range(NCORES)), trace=trace)
    out = np.concatenate(
        [res.results[i]["out"].reshape(BPC, S, H) for i in range(NCORES)], axis=0)
    return out, res


def kernel(**inputs):
    return run(inputs)[0]
